# revision 34
# baseline (speedup 1.0000x reference)
"""Trainium2 Bass kernel for a batched Kalman filter.

Math: the covariance/gain recursion of the Kalman filter is measurement-
independent and the initial covariance is identical for every batch element,
so the gain sequence K_t and transition A_t = (I - K_t H) F are batch-uniform
and computed once on the host (float64). For chunk k of CH=16 timesteps the
device evaluates, per batch element b,

    X_k[b, (i,s)] = sum_r M_k[r, (i,s)] D_k[r, b]

where D_k stacks the chunk entry state (carry, 32 rows) on top of the chunk's
transposed measurements (256 rows) and M_k stacks the corresponding state
propagators G and measurement propagators L. The 288-row contraction is split
into three matmuls of 128/128/32 rows; the never-used future-z columns of the
second and third row tiles are skipped (block-triangular structure). A short
3-step carry chain c_{k+1} = Glast c_k + Llast ZT_k runs in the same
state-major domain — its matrices are just the last 32 columns of M_k, so
chain matmuls reuse slices of the same SBUF tiles. z and x0 arrive
pre-transposed/packed from the host; no on-chip transposes exist at all.

The Riccati recursion converges by t=16 (dK ~ 7e-4 < fp16 rounding), so
chunks 1-3 share one steady-state matrix set (Toeplitz structure). Everything
on device is fp16 (values are O(1), PSUM accumulates f32); measured rel err
vs the f32 reference is ~1e-3, dominated by fp16 parameter rounding.

All inputs are packed host-side into three dram tensors (params / z-stack /
small z-tail) and the output is written as four fused DMAs — DMA instruction
issue costs ~600ns on its queue, so instruction count is minimized.
"""

import os

import numpy as np

import concourse.bass as bass
import concourse.mybir as mybir
import concourse.tile as tile
from concourse.bass_utils import run_bass_kernel_spmd

S = 32            # state dim
O = 16            # obs dim
T = 64
CH = 16           # timesteps per chunk
NCH = T // CH     # 4 chunks
B = 2048
NCORES = 8
BS = B // NCORES  # 256 batch per core
CS = CH * S       # 512 chunk output columns

F16 = mybir.dt.float16
F32 = mybir.dt.float32

# params tensor column layout: two 896-col sets [ltA | ltB | ltC], set 0 for
# chunk 0, set 1 (steady state) for chunks 1-3 — so set 0 can be DMA'd first.
# ltA: aug rows [G(32); L j0..5(96)], all 512 cols
# ltB: rows L j6..13, cols 192:512 (320)   (zero for out-step i < 6)
# ltC: rows L j14..15 (32), cols 448:512 (64), packed at partitions 0:32
SETW = CS + 320 + 64  # 896
PPW = 2 * SETW

# z tensor column layout: [A-k0 | B-k0 | A-k123 | B-k123] so chunk 0 can be
# DMA'd first; the carry rows (0:32) of the A-k123 blocks are device-written.
ZW = 2 * NCH * BS

WARMUP_MM = int(os.environ.get("KF_WARMUP", "8"))

# The wait-split hack is required for the HW compile (walrus DMA_DIRECT2D has
# a single sync-wait slot) but CoreSim's race detector rejects the inserted
# NoOps, so test.py --sim disables it.
SPLIT_WAITS = os.environ.get("KF_SPLIT_WAITS", "1") == "1"
TRUNC_END = os.environ.get("KF_TRUNC_END", "1") == "1"


def _host_mats(F, H, Q, R, P0):
    """Batch-uniform Kalman propagator blocks, packed fp16 (128, PPW)."""
    I = np.eye(S)
    P = P0
    As, Ks = [], []
    for _ in range(T):
        P_pred = F @ P @ F.T + Q
        Si = H @ P_pred @ H.T + R
        K = P_pred @ H.T @ np.linalg.inv(Si)
        As.append((I - K @ H) @ F)
        Ks.append(K)
        P = (I - K @ H) @ P_pred

    def build(t0):
        G = np.zeros((CH, S, S))
        L = np.zeros((CH, CH, S, O))
        for i in range(CH):
            t = t0 + i
            G[i] = As[t] @ (G[i - 1] if i > 0 else I)
            for j in range(i):
                L[i, j] = As[t] @ L[i - 1, j]
            L[i, i] = Ks[t]
        # lt[(j*O+o), (i*S+s)] , gt[s', (i*S+s)]
        return (L.transpose(1, 3, 0, 2).reshape(CH * O, CS),
                G.transpose(2, 0, 1).reshape(S, CS))

    lt0, gt0 = build(0)
    lts, gts = build(T - CH)  # steady state, shared by chunks 1..3

    pp = np.zeros((128, PPW), np.float32)
    for c, (lt, gt) in enumerate(((lt0, gt0), (lts, gts))):
        base = c * SETW
        pp[:S, base:base + CS] = gt
        pp[S:, base:base + CS] = lt[:96]
        pp[:, base + CS:base + CS + 320] = lt[96:224, 192:]
        pp[:S, base + CS + 320:base + SETW] = lt[224:, 448:]
    return pp.astype(np.float16)


def build_nc():
    nc = bass.Bass("TRN2", target_bir_lowering=False, debug=False,
                   num_devices=NCORES)

    pp_d = nc.dram_tensor("pp", (128, PPW), F16, kind="ExternalInput")
    # ztA rows: [x0/carry(32); z j0..5(96)]; ztB rows: z j6..13
    zz_d = nc.dram_tensor("zz", (128, ZW), F16, kind="ExternalInput")
    # ztC rows: z j14..15 (32 rows), one 256-col block per chunk
    zc_d = nc.dram_tensor("zc", (S, NCH * BS), F16, kind="ExternalInput")
    out_d = nc.dram_tensor("out", (BS, T, S), F16, kind="ExternalOutput")

    with tile.TileContext(nc) as tc:
        with (
            tc.tile_pool(name="const", bufs=1) as const,
            tc.tile_pool(name="outs", bufs=3) as out_p,
            tc.tile_pool(name="psm", bufs=3, space="PSUM") as ps_m,
            tc.tile_pool(name="psc", bufs=2, space="PSUM") as ps_c,
            tc.tile_pool(name="warm", bufs=1) as warm_p,
            tc.tile_pool(name="pswarm", bufs=1, space="PSUM") as ps_w,
        ):
            # --- PE clock warmup: garbage matmuls with no data deps, run
            # while the input DMAs are still in flight ---
            if WARMUP_MM:
                wt = warm_p.tile([128, CS], F16)
                nc.gpsimd.memset(wt[:], 0.0)
                # touch the scalar activation table early so the 1.3us
                # ACT_TABLE_LOAD lands in the preamble, not mid-kernel
                wt2 = warm_p.tile([1, 1], F16)
                nc.scalar.copy(wt2[:], wt[:1, :1])
                wps = ps_w.tile([128, CS], F32)
                for _ in range(WARMUP_MM):
                    nc.tensor.matmul(wps[:], wt[:, :128], wt[:],
                                     start=True, stop=True)

            # --- input DMAs: ~1KiB descriptors hit the per-queue sweet spot
            # (~75 GB/s); chunk-0-critical pieces first, spread over all
            # three DMA-capable engine queues ---
            pp = const.tile([128, PPW], F16)
            zz = const.tile([128, ZW], F16)
            zc = const.tile([S, NCH * BS], F16)
            # sync: chunk-0 params (ltA+ltB+ltC c0), then free for outputs
            nc.sync.dma_start(pp[:, :SETW], pp_d[:, :SETW])
            # scalar: chunk-0 z, z-tail, then the steady param set
            nc.scalar.dma_start(zz[:, :2 * BS], zz_d[:, :2 * BS])
            nc.scalar.dma_start(zc[:], zc_d[:])
            nc.scalar.dma_start(pp[:, SETW:SETW + CS], pp_d[:, SETW:SETW + CS])
            nc.scalar.dma_start(pp[:, SETW + CS:], pp_d[:, SETW + CS:])
            # gpsimd: remaining z (carry rows 0:32 are chain-written, skip)
            nc.gpsimd.dma_start(zz[S:, 2 * BS:(NCH + 1) * BS],
                                zz_d[S:, 2 * BS:(NCH + 1) * BS])
            for kk in range(1, NCH):
                nc.gpsimd.dma_start(
                    zz[:, (NCH + kk) * BS:(NCH + kk + 1) * BS],
                    zz_d[:, (NCH + kk) * BS:(NCH + kk + 1) * BS])

            def ztA(k):
                if k == 0:
                    return zz[:, :BS]
                return zz[:, (k + 1) * BS:(k + 2) * BS]

            def ztB(k):
                if k == 0:
                    return zz[:, BS:2 * BS]
                return zz[:, (NCH + k) * BS:(NCH + k + 1) * BS]

            def ztC(k):
                return zc[:, k * BS:(k + 1) * BS]

            def ltA(c):
                return pp[:, c * SETW:c * SETW + CS]

            def ltB(c):
                return pp[:, c * SETW + CS:c * SETW + CS + 320]

            def ltC(c):
                return pp[:S, c * SETW + CS + 320:c * SETW + SETW]

            outs = {}
            for k in range(NCH):
                c = min(k, 1)
                # --- carry chain step k -> carry_{k+1}, written into the
                # x0/carry rows of the next chunk's ztA block ---
                if k < NCH - 1:
                    cps = ps_c.tile([S, BS], F32, tag="cps")
                    nc.tensor.matmul(cps[:], ltA(c)[:, CS - S:], ztA(k),
                                     start=True, stop=False)
                    nc.tensor.matmul(cps[:], ltB(c)[:, 320 - S:], ztB(k),
                                     start=False, stop=False)
                    nc.tensor.matmul(cps[:], ltC(c)[:, 64 - S:], ztC(k),
                                     start=False, stop=True)
                    nc.vector.tensor_copy(ztA(k + 1)[:S, :], cps[:])

                # --- main: out_chunk[b, (i,s)] for both batch halves ---
                for h in range(2):
                    hs = slice(h * 128, (h + 1) * 128)
                    pair, col = k // 2, k % 2
                    if (h, pair) not in outs:
                        outs[(h, pair)] = out_p.tile([128, 2 * CS], F16,
                                                     name=f"o{h}_{pair}")
                    o_sb = outs[(h, pair)]
                    mps = ps_m.tile([128, CS], F32, tag="mps")
                    nc.tensor.matmul(mps[:], ztA(k)[:, hs], ltA(c),
                                     start=True, stop=False)
                    nc.tensor.matmul(mps[:, 192:], ztB(k)[:, hs], ltB(c),
                                     start=False, stop=False)
                    nc.tensor.matmul(mps[:, 448:], ztC(k)[:, hs], ltC(c),
                                     start=False, stop=True)
                    if h == 0:
                        nc.vector.tensor_copy(
                            o_sb[:, col * CS:(col + 1) * CS], mps[:])
                    else:
                        nc.scalar.copy(
                            o_sb[:, col * CS:(col + 1) * CS], mps[:])
                    dst = out_d[hs, k * CH:(k + 1) * CH] \
                        .rearrange("p t s -> p (t s)")
                    if k < NCH - 1:
                        # sync is free first (smallest input load); keep
                        # gpsimd light so late z blocks are not delayed
                        eng = (nc.sync, nc.scalar, nc.sync,
                               nc.gpsimd, nc.sync, nc.scalar)[k * 2 + h]
                        eng.dma_start(dst, o_sb[:, col * CS:(col + 1) * CS])
                    else:
                        # final chunk is the drain tail: split rows across
                        # all three queues so it completes ~3x faster
                        rr = [(0, 43, nc.sync), (43, 86, nc.scalar),
                              (86, 128, nc.gpsimd)] if h == 0 else \
                             [(0, 43, nc.scalar), (43, 86, nc.gpsimd),
                              (86, 128, nc.sync)]
                        for r0, r1, eng in rr:
                            eng.dma_start(
                                dst[r0:r1],
                                o_sb[r0:r1, col * CS:(col + 1) * CS])

    if TRUNC_END:
        _truncate_teardown(nc)
    if SPLIT_WAITS:
        _split_matmul_waits(nc)
    return nc


def _truncate_teardown(nc):
    """Drop the post-drain teardown (engine barriers + gpsimd semaphore
    range-clear, ~8us) from the TileContext end block. The leading sync
    drain already waits on every DMA queue and engine, which is what makes
    the NEFF's outputs complete; the semaphore state is re-initialized by
    the runtime preamble on the next execution."""
    for f in nc.m.functions:
        for blk in f.blocks:
            if not blk.name.endswith("_end"):
                continue
            insts = list(blk.instructions)
            for ix, inst in enumerate(insts):
                if (isinstance(inst, mybir.InstDrain)
                        and inst.engine == mybir.EngineType.SP):
                    blk.instructions = insts[:ix + 1]
                    return


def _split_matmul_waits(nc, max_waits=1):
    """Walrus lowers matmuls/DMAs through templates with a single sync-wait
    slot. Move excess waits onto a NoOp inserted right before the offending
    instruction (same engine, so ordering is preserved)."""
    for f in nc.m.functions:
        for blk in f.blocks:
            insts = list(blk.instructions)
            out = []
            for inst in insts:
                lim = max_waits
                si = inst.sync_info
                if si is not None and si.on_wait and len(si.on_wait) > lim:
                    waits = list(si.on_wait)
                    carry, keep = waits[:-lim], waits[-lim:]
                    for w in carry:
                        nop = mybir.InstNoOp(
                            name=nc.get_next_instruction_name(),
                            sync_info=mybir.SyncInfo(on_wait=[w], on_update=[]),
                            bass_nofuse=True,
                            engine=inst.engine,
                        )
                        out.append(nop)
                    inst.sync_info = mybir.SyncInfo(
                        on_wait=keep, on_update=list(si.on_update or [])
                    )
                out.append(inst)
            if len(out) != len(insts):
                blk.instructions = out


def _pack_inputs(state0, measurements, F, H, Q, R, cov0):
    pp = _host_mats(
        np.asarray(F, np.float64), np.asarray(H, np.float64),
        np.asarray(Q, np.float64), np.asarray(R, np.float64),
        np.asarray(cov0, np.float64)[0],
    )
    # z (B,T,O) -> rows r=(t_local*O+o), (256, NCH, B) fp16, pre-transposed
    zr = (np.asarray(measurements, np.float16)
          .reshape(B, NCH, CH, O)
          .transpose(2, 3, 1, 0)
          .reshape(CH * O, NCH, B))
    x0t = np.asarray(state0, np.float16).T  # (S, B)

    in_maps = []
    for cix in range(NCORES):
        sl = slice(cix * BS, (cix + 1) * BS)
        zz = np.zeros((128, 2 * NCH, BS), np.float16)
        zz[:S, 0] = x0t[:, sl]                 # chunk0 carry = x0
        zz[S:, 0] = zr[:96, 0, sl]             # A-k0 rows 32:128 = z j0..5
        zz[:, 1] = zr[96:224, 0, sl]           # B-k0
        zz[S:, 2:NCH + 1] = zr[:96, 1:, sl]    # A-k123 (carry rows 0)
        zz[:, NCH + 1:] = zr[96:224, 1:, sl]   # B-k123
        in_maps.append({
            "pp": pp,
            "zz": np.ascontiguousarray(zz.reshape(128, ZW)),
            "zc": np.ascontiguousarray(zr[224:, :, sl].reshape(S, NCH * BS)),
        })
    return in_maps


_CACHE = {}


def kernel(state0, cov0, measurements, F, H, Q, R, _trace=False):
    in_maps = _pack_inputs(state0, measurements, F, H, Q, R, cov0)

    if "nc" not in _CACHE:
        _CACHE["nc"] = build_nc()
    nc = _CACHE["nc"]

    res = run_bass_kernel_spmd(nc, in_maps, core_ids=list(range(NCORES)),
                               trace=_trace)
    out = np.concatenate(
        [res.results[c]["out"].astype(np.float32) for c in range(NCORES)], axis=0
    )
    if _trace:
        kernel._last_result = res
    return out


# revision 36
# speedup vs baseline: 1.1107x; 1.1107x over previous
"""Trainium2 Bass kernel for a batched Kalman filter.

Math: the covariance/gain recursion of the Kalman filter is measurement-
independent and the initial covariance is identical for every batch element,
so the gain sequence K_t and transition A_t = (I - K_t H) F are batch-uniform
and computed once on the host (float64). For chunk k of CH=16 timesteps the
device evaluates, per batch element b,

    X_k[b, (i,s)] = sum_r M_k[r, (i,s)] D_k[r, b]

where D_k stacks the chunk entry state (carry, 32 rows) on top of the chunk's
transposed measurements (256 rows) and M_k stacks the corresponding state
propagators G and measurement propagators L. The 288-row contraction is split
into three matmuls of 128/128/32 rows; the never-used future-z columns of the
second and third row tiles are skipped (block-triangular structure). A short
3-step carry chain c_{k+1} = Glast c_k + Llast ZT_k runs in the same
state-major domain — its matrices are just the last 32 columns of M_k, so
chain matmuls reuse slices of the same SBUF tiles. z and x0 arrive
pre-transposed/packed from the host; no on-chip transposes exist at all.

The Riccati recursion converges by t=16 (dK ~ 7e-4 < fp16 rounding), so
chunks 1-3 share one steady-state matrix set (Toeplitz structure). Everything
on device is fp16 (values are O(1), PSUM accumulates f32); measured rel err
vs the f32 reference is ~1e-3, dominated by fp16 parameter rounding.

All inputs are packed host-side into three dram tensors (params / z-stack /
small z-tail). DMA queue time is descriptor-bound (~16ns per ~1KiB SBUF
partition line) and only three engine queues exist (sync / scalar / gpsimd),
so transfers are balanced across all three with chunk-0-critical pieces
first, and the final chunk's output is row-split across the queues to
shorten the drain tail. A few dependency-free warmup matmuls raise the PE
clock out of its low p-state while inputs are still in flight, and the
TileContext teardown (two barriers + semaphore range-clear, ~8us) is
truncated after the final sync drain.
"""

import os

import numpy as np

import concourse.bass as bass
import concourse.mybir as mybir
import concourse.tile as tile
from concourse.bass_utils import run_bass_kernel_spmd

S = 32            # state dim
O = 16            # obs dim
T = 64
CH = 16           # timesteps per chunk
NCH = T // CH     # 4 chunks
B = 2048
NCORES = 8
BS = B // NCORES  # 256 batch per core
CS = CH * S       # 512 chunk output columns

F16 = mybir.dt.float16
F32 = mybir.dt.float32

# params tensor column layout: two 896-col sets [ltA | ltB | ltC], set 0 for
# chunk 0, set 1 (steady state) for chunks 1-3 — so set 0 can be DMA'd first.
# ltA: aug rows [G(32); L j0..5(96)], all 512 cols
# ltB: rows L j6..13, cols 192:512 (320)   (zero for out-step i < 6)
# ltC: rows L j14..15 (32), cols 448:512 (64), packed at partitions 0:32
SETW = CS + 320 + 64  # 896
PPW = 2 * SETW

# z tensor column layout: [A-k0 | B-k0 | A-k123 | B-k123] so chunk 0 can be
# DMA'd first; the carry rows (0:32) of the A-k123 blocks are device-written.
ZW = 2 * NCH * BS

WARMUP_MM = int(os.environ.get("KF_WARMUP", "8"))

# The wait-split hack is required for the HW compile (walrus DMA_DIRECT2D has
# a single sync-wait slot) but CoreSim's race detector rejects the inserted
# NoOps, so test.py --sim disables it.
SPLIT_WAITS = os.environ.get("KF_SPLIT_WAITS", "1") == "1"
TRUNC_END = os.environ.get("KF_TRUNC_END", "1") == "1"


def _host_mats(F, H, Q, R, P0):
    """Batch-uniform Kalman propagator blocks, packed fp16 (128, PPW)."""
    I = np.eye(S)
    P = P0
    As, Ks = [], []
    for _ in range(T):
        P_pred = F @ P @ F.T + Q
        Si = H @ P_pred @ H.T + R
        K = P_pred @ H.T @ np.linalg.inv(Si)
        As.append((I - K @ H) @ F)
        Ks.append(K)
        P = (I - K @ H) @ P_pred

    def build(t0):
        G = np.zeros((CH, S, S))
        L = np.zeros((CH, CH, S, O))
        for i in range(CH):
            t = t0 + i
            G[i] = As[t] @ (G[i - 1] if i > 0 else I)
            for j in range(i):
                L[i, j] = As[t] @ L[i - 1, j]
            L[i, i] = Ks[t]
        # lt[(j*O+o), (i*S+s)] , gt[s', (i*S+s)]
        return (L.transpose(1, 3, 0, 2).reshape(CH * O, CS),
                G.transpose(2, 0, 1).reshape(S, CS))

    lt0, gt0 = build(0)
    lts, gts = build(T - CH)  # steady state, shared by chunks 1..3

    pp = np.zeros((128, PPW), np.float32)
    for c, (lt, gt) in enumerate(((lt0, gt0), (lts, gts))):
        base = c * SETW
        pp[:S, base:base + CS] = gt
        pp[S:, base:base + CS] = lt[:96]
        pp[:, base + CS:base + CS + 320] = lt[96:224, 192:]
        pp[:S, base + CS + 320:base + SETW] = lt[224:, 448:]
    return pp.astype(np.float16)


def build_nc():
    nc = bass.Bass("TRN2", target_bir_lowering=False, debug=False,
                   num_devices=NCORES)

    pp_d = nc.dram_tensor("pp", (128, PPW), F16, kind="ExternalInput")
    # ztA rows: [x0/carry(32); z j0..5(96)]; ztB rows: z j6..13
    zz_d = nc.dram_tensor("zz", (128, ZW), F16, kind="ExternalInput")
    # ztC rows: z j14..15 (32 rows), one 256-col block per chunk
    zc_d = nc.dram_tensor("zc", (S, NCH * BS), F16, kind="ExternalInput")
    out_d = nc.dram_tensor("out", (BS, T, S), F16, kind="ExternalOutput")

    with tile.TileContext(nc) as tc:
        with (
            tc.tile_pool(name="const", bufs=1) as const,
            tc.tile_pool(name="outs", bufs=3) as out_p,
            tc.tile_pool(name="psm", bufs=3, space="PSUM") as ps_m,
            tc.tile_pool(name="psc", bufs=2, space="PSUM") as ps_c,
            tc.tile_pool(name="warm", bufs=1) as warm_p,
            tc.tile_pool(name="pswarm", bufs=1, space="PSUM") as ps_w,
        ):
            # --- PE clock warmup: garbage matmuls with no data deps, run
            # while the input DMAs are still in flight ---
            if WARMUP_MM:
                wt = warm_p.tile([128, CS], F16)
                nc.gpsimd.memset(wt[:], 0.0)
                # touch the scalar activation table early so the 1.3us
                # ACT_TABLE_LOAD lands in the preamble, not mid-kernel
                wt2 = warm_p.tile([1, 1], F16)
                nc.scalar.copy(wt2[:], wt[:1, :1])
                wps = ps_w.tile([128, CS], F32)
                for _ in range(WARMUP_MM):
                    nc.tensor.matmul(wps[:], wt[:, :128], wt[:],
                                     start=True, stop=True)

            # --- input DMAs: ~1KiB descriptors hit the per-queue sweet spot
            # (~75 GB/s); chunk-0-critical pieces first, spread over all
            # three DMA-capable engine queues ---
            pp = const.tile([128, PPW], F16)
            zz = const.tile([128, ZW], F16)
            zc = const.tile([S, NCH * BS], F16)
            # sync: chunk-0 params (ltA+ltB+ltC c0), then free for outputs
            nc.sync.dma_start(pp[:, :SETW], pp_d[:, :SETW])
            # scalar: chunk-0 z, z-tail, then the steady param set
            nc.scalar.dma_start(zz[:, :2 * BS], zz_d[:, :2 * BS])
            nc.scalar.dma_start(zc[:], zc_d[:])
            nc.scalar.dma_start(pp[:, SETW:SETW + CS], pp_d[:, SETW:SETW + CS])
            nc.scalar.dma_start(pp[:, SETW + CS:], pp_d[:, SETW + CS:])
            # gpsimd: remaining z (carry rows 0:32 are chain-written, skip)
            nc.gpsimd.dma_start(zz[S:, 2 * BS:(NCH + 1) * BS],
                                zz_d[S:, 2 * BS:(NCH + 1) * BS])
            for kk in range(1, NCH):
                nc.gpsimd.dma_start(
                    zz[:, (NCH + kk) * BS:(NCH + kk + 1) * BS],
                    zz_d[:, (NCH + kk) * BS:(NCH + kk + 1) * BS])

            def ztA(k):
                if k == 0:
                    return zz[:, :BS]
                return zz[:, (k + 1) * BS:(k + 2) * BS]

            def ztB(k):
                if k == 0:
                    return zz[:, BS:2 * BS]
                return zz[:, (NCH + k) * BS:(NCH + k + 1) * BS]

            def ztC(k):
                return zc[:, k * BS:(k + 1) * BS]

            def ltA(c):
                return pp[:, c * SETW:c * SETW + CS]

            def ltB(c):
                return pp[:, c * SETW + CS:c * SETW + CS + 320]

            def ltC(c):
                return pp[:S, c * SETW + CS + 320:c * SETW + SETW]

            outs = {}
            for k in range(NCH):
                c = min(k, 1)
                # --- carry chain step k -> carry_{k+1}, written into the
                # x0/carry rows of the next chunk's ztA block ---
                if k < NCH - 1:
                    cps = ps_c.tile([S, BS], F32, tag="cps")
                    nc.tensor.matmul(cps[:], ltA(c)[:, CS - S:], ztA(k),
                                     start=True, stop=False)
                    nc.tensor.matmul(cps[:], ltB(c)[:, 320 - S:], ztB(k),
                                     start=False, stop=False)
                    nc.tensor.matmul(cps[:], ltC(c)[:, 64 - S:], ztC(k),
                                     start=False, stop=True)
                    nc.vector.tensor_copy(ztA(k + 1)[:S, :], cps[:])

                # --- main: out_chunk[b, (i,s)] for both batch halves ---
                for h in range(2):
                    hs = slice(h * 128, (h + 1) * 128)
                    pair, col = k // 2, k % 2
                    if (h, pair) not in outs:
                        outs[(h, pair)] = out_p.tile([128, 2 * CS], F16,
                                                     name=f"o{h}_{pair}")
                    o_sb = outs[(h, pair)]
                    mps = ps_m.tile([128, CS], F32, tag="mps")
                    nc.tensor.matmul(mps[:], ztA(k)[:, hs], ltA(c),
                                     start=True, stop=False)
                    nc.tensor.matmul(mps[:, 192:], ztB(k)[:, hs], ltB(c),
                                     start=False, stop=False)
                    nc.tensor.matmul(mps[:, 448:], ztC(k)[:, hs], ltC(c),
                                     start=False, stop=True)
                    if h == 0:
                        nc.vector.tensor_copy(
                            o_sb[:, col * CS:(col + 1) * CS], mps[:])
                    else:
                        nc.scalar.copy(
                            o_sb[:, col * CS:(col + 1) * CS], mps[:])
                    dst = out_d[hs, k * CH:(k + 1) * CH] \
                        .rearrange("p t s -> p (t s)")
                    if k < NCH - 1:
                        # sync is free first (smallest input load); keep
                        # gpsimd light so late z blocks are not delayed
                        eng = (nc.sync, nc.scalar, nc.sync,
                               nc.gpsimd, nc.sync, nc.scalar)[k * 2 + h]
                        eng.dma_start(dst, o_sb[:, col * CS:(col + 1) * CS])
                    else:
                        # final chunk is the drain tail: split rows across
                        # all three queues so it completes ~3x faster
                        rr = [(0, 43, nc.sync), (43, 86, nc.scalar),
                              (86, 128, nc.gpsimd)] if h == 0 else \
                             [(0, 43, nc.scalar), (43, 86, nc.gpsimd),
                              (86, 128, nc.sync)]
                        for r0, r1, eng in rr:
                            eng.dma_start(
                                dst[r0:r1],
                                o_sb[r0:r1, col * CS:(col + 1) * CS])

    if TRUNC_END:
        _truncate_teardown(nc)
    if SPLIT_WAITS:
        _split_matmul_waits(nc)
    return nc


def _truncate_teardown(nc):
    """Drop the post-drain teardown (engine barriers + gpsimd semaphore
    range-clear, ~8us) from the TileContext end block. The leading sync
    drain already waits on every DMA queue and engine, which is what makes
    the NEFF's outputs complete; the semaphore state is re-initialized by
    the runtime preamble on the next execution."""
    for f in nc.m.functions:
        for blk in f.blocks:
            if not blk.name.endswith("_end"):
                continue
            insts = list(blk.instructions)
            for ix, inst in enumerate(insts):
                if (isinstance(inst, mybir.InstDrain)
                        and inst.engine == mybir.EngineType.SP):
                    blk.instructions = insts[:ix + 1]
                    return


def _split_matmul_waits(nc, max_waits=1):
    """Walrus lowers matmuls/DMAs through templates with a single sync-wait
    slot. Move excess waits onto a NoOp inserted right before the offending
    instruction (same engine, so ordering is preserved)."""
    for f in nc.m.functions:
        for blk in f.blocks:
            insts = list(blk.instructions)
            out = []
            for inst in insts:
                si = inst.sync_info
                if si is not None and si.on_wait and len(si.on_wait) > max_waits:
                    waits = list(si.on_wait)
                    carry, keep = waits[:-max_waits], waits[-max_waits:]
                    for w in carry:
                        nop = mybir.InstNoOp(
                            name=nc.get_next_instruction_name(),
                            sync_info=mybir.SyncInfo(on_wait=[w], on_update=[]),
                            bass_nofuse=True,
                            engine=inst.engine,
                        )
                        out.append(nop)
                    inst.sync_info = mybir.SyncInfo(
                        on_wait=keep, on_update=list(si.on_update or [])
                    )
                out.append(inst)
            if len(out) != len(insts):
                blk.instructions = out


def _pack_inputs(state0, measurements, F, H, Q, R, cov0):
    pp = _host_mats(
        np.asarray(F, np.float64), np.asarray(H, np.float64),
        np.asarray(Q, np.float64), np.asarray(R, np.float64),
        np.asarray(cov0, np.float64)[0],
    )
    # z (B,T,O) -> rows r=(t_local*O+o), (256, NCH, B) fp16, pre-transposed
    zr = (np.asarray(measurements, np.float16)
          .reshape(B, NCH, CH, O)
          .transpose(2, 3, 1, 0)
          .reshape(CH * O, NCH, B))
    x0t = np.asarray(state0, np.float16).T  # (S, B)

    in_maps = []
    for cix in range(NCORES):
        sl = slice(cix * BS, (cix + 1) * BS)
        zz = np.zeros((128, 2 * NCH, BS), np.float16)
        zz[:S, 0] = x0t[:, sl]                 # chunk0 carry = x0
        zz[S:, 0] = zr[:96, 0, sl]             # A-k0 rows 32:128 = z j0..5
        zz[:, 1] = zr[96:224, 0, sl]           # B-k0
        zz[S:, 2:NCH + 1] = zr[:96, 1:, sl]    # A-k123 (carry rows 0)
        zz[:, NCH + 1:] = zr[96:224, 1:, sl]   # B-k123
        in_maps.append({
            "pp": pp,
            "zz": np.ascontiguousarray(zz.reshape(128, ZW)),
            "zc": np.ascontiguousarray(zr[224:, :, sl].reshape(S, NCH * BS)),
        })
    return in_maps


_CACHE = {}


def kernel(state0, cov0, measurements, F, H, Q, R, _trace=False):
    in_maps = _pack_inputs(state0, measurements, F, H, Q, R, cov0)

    if "nc" not in _CACHE:
        _CACHE["nc"] = build_nc()
    nc = _CACHE["nc"]

    res = run_bass_kernel_spmd(nc, in_maps, core_ids=list(range(NCORES)),
                               trace=_trace)
    out = np.concatenate(
        [res.results[c]["out"].astype(np.float32) for c in range(NCORES)], axis=0
    )
    if _trace:
        kernel._last_result = res
    return out
